# revision 6
# baseline (speedup 1.0000x reference)
"""Trainium2 Bass kernel for CachedEHREmbeddings (embedding_lookup).

Strategy (data-parallel over batch, 4 batch rows -> 8192 tokens/core):
  - Host caches the projected word table W2 = W_word @ lin_W[:768] + lin_b
    (exact algebra: word @ L_w == (W_word @ L_w)[ids]); device gathers fp16
    rows of W2 instead of running the K=833 matmul.
  - Per 128-token tile: sin(dt*w+phi)/sin(age*w+phi) on ACT, transposed on
    PE; pre-tanh = W2[ids] (ident matmul into PSUM) + sinsT.T @ L_s.
  - Post-tanh adds (type+seg fused into a 27-row table, order 512-row
    table) via one-hot fp16 matmuls; tanh accumulated into the same PSUM
    bank with an identity matmul.
  - LayerNorm stats via ACT accum_out (tanh sum + table row-sums as a
    769th matmul column; Square accum for E[x^2]); normalize on DVE.
  - Output staged in SBUF, written 8 tiles per HWDGE DMA.
"""

import sys

for _p in ("/opt/trn_rl_repo",):
    if _p not in sys.path:
        sys.path.insert(0, _p)

import numpy as np

import concourse.bass as bass
import concourse.bacc as bacc
import concourse.tile as tile
from concourse import mybir
from concourse.bass import IndirectOffsetOnAxis
from concourse.bass_utils import run_bass_kernel_spmd

# Problem constants (hardcoded per contract)
V, H, T = 32000, 768, 32
TYPES, MAX_VISITS, SEGS = 9, 512, 3
B, S = 32, 2048
B_PER = B // 8
N_CORES = 8
TOK = B_PER * S                 # 8192 tokens per core
P = 128
NTILES = TOK // P               # 64
NTS = TYPES * SEGS              # 27 fused type*3+seg rows
GB = 1                          # tiles per gather instruction
OB = 8                          # tiles per output DMA

F32 = mybir.dt.float32
F16 = mybir.dt.float16
I32 = mybir.dt.int32
AF = mybir.ActivationFunctionType
OP = mybir.AluOpType


def build_nc(apply_gb: bool):
    nc = bacc.Bacc("TRN2", target_bir_lowering=False, debug=False,
                   num_devices=N_CORES)

    meta_i_d = nc.declare_dram_parameter("meta_i", [P, NTILES], I32, isOutput=False)
    meta_f_d = nc.declare_dram_parameter("meta_f", [P, NTILES, 4], F32, isOutput=False)
    w2_d = nc.declare_dram_parameter("W2", [V, H // 2], F32, isOutput=False)
    wcat_d = nc.declare_dram_parameter("Wcat", [MAX_VISITS, H + 1], F16, isOutput=False)
    wts_d = nc.declare_dram_parameter("Wts", [NTS, H + 1], F16, isOutput=False)
    ls_d = nc.declare_dram_parameter("Ls", [2 * T, H], F16, isOutput=False)
    id32_d = nc.declare_dram_parameter("id32", [P, P], F32, isOutput=False)
    id16_d = nc.declare_dram_parameter("id16", [P, P], F16, isOutput=False)
    wb_d = nc.declare_dram_parameter("wb", [P, 2 * T], F32, isOutput=False)
    phib_d = nc.declare_dram_parameter("phib", [P, 2 * T], F32, isOutput=False)
    iord_d = nc.declare_dram_parameter("iord", [P, 4], F32, isOutput=False)
    its_d = nc.declare_dram_parameter("its", [NTS, 1], F32, isOutput=False)
    if apply_gb:
        ln_g_d = nc.declare_dram_parameter("ln_g", [P, H], F32, isOutput=False)
        ln_b_d = nc.declare_dram_parameter("ln_beta", [P, H], F32, isOutput=False)
    out_d = nc.declare_dram_parameter("out", [P, NTILES * H], F32, isOutput=True)

    with tile.TileContext(nc) as tc:
        with (
            tc.tile_pool(name="singles", bufs=1) as singles,
            tc.tile_pool(name="wgp", bufs=3) as wgp,
            tc.tile_pool(name="blkp", bufs=3) as blkp,
            tc.tile_pool(name="orp", bufs=3) as orp,
            tc.tile_pool(name="sbp", bufs=3) as sbp,
            tc.tile_pool(name="otp", bufs=3) as otp,
            tc.tile_pool(name="ohp", bufs=3) as ohp,
            tc.tile_pool(name="thp", bufs=3) as thp,
            tc.tile_pool(name="sqp", bufs=2) as sqp,
            tc.tile_pool(name="scl", bufs=4) as scl,
            tc.tile_pool(name="ostp", bufs=2) as ostp,
            tc.tile_pool(name="psap", bufs=2, space="PSUM") as psap,
            tc.tile_pool(name="psbp", bufs=2, space="PSUM") as psbp,
        ):
            # ---- constants ----
            meta_i = singles.tile([P, NTILES], I32, tag="meta_i")
            nc.sync.dma_start(out=meta_i[:], in_=meta_i_d[:, :])
            meta_f = singles.tile([P, NTILES, 4], F32, tag="meta_f")
            nc.sync.dma_start(out=meta_f[:], in_=meta_f_d[:, :, :])
            wcat = singles.tile([P, 4, H + 1], F16, tag="wcat")
            for c in range(4):
                nc.sync.dma_start(out=wcat[:, c, :],
                                  in_=wcat_d[c * P:(c + 1) * P, :])
            wts = singles.tile([NTS, H + 1], F16, tag="wts")
            nc.sync.dma_start(out=wts[:], in_=wts_d[:, :])
            ls = singles.tile([2 * T, H], F16, tag="ls")
            nc.sync.dma_start(out=ls[:], in_=ls_d[:, :])
            id32 = singles.tile([P, P], F32, tag="id32")
            nc.sync.dma_start(out=id32[:], in_=id32_d[:, :])
            id16 = singles.tile([P, P], F16, tag="id16")
            nc.sync.dma_start(out=id16[:], in_=id16_d[:, :])
            wb = singles.tile([P, 2 * T], F32, tag="wb")
            nc.sync.dma_start(out=wb[:], in_=wb_d[:, :])
            phib = singles.tile([P, 2 * T], F32, tag="phib")
            nc.sync.dma_start(out=phib[:], in_=phib_d[:, :])
            iord = singles.tile([P, 4], F32, tag="iord")
            nc.sync.dma_start(out=iord[:], in_=iord_d[:, :])
            its = singles.tile([NTS, 1], F32, tag="its")
            nc.sync.dma_start(out=its[:], in_=its_d[:, :])
            if apply_gb:
                g_sb = singles.tile([P, H], F32, tag="g")
                nc.sync.dma_start(out=g_sb[:], in_=ln_g_d[:, :])
                b_sb = singles.tile([P, H], F32, tag="b")
                nc.sync.dma_start(out=b_sb[:], in_=ln_b_d[:, :])

            ostage = None
            for i in range(NTILES):
                # ---- word gather: fp16 W2 rows via an f32-typed view
                # (f16-typed indirect DMA lowers to 256B descriptors; the
                # f32 view keeps whole-row descriptors)
                wg = wgp.tile([P, H], F16, tag="wg")
                nc.gpsimd.indirect_dma_start(
                    out=wg[:, :].bitcast(F32),
                    out_offset=None,
                    in_=w2_d[:, :],
                    in_offset=IndirectOffsetOnAxis(
                        ap=meta_i[:, i:i + 1], axis=0),
                )

                # ---- sin features + replicated indices ----
                blk = blkp.tile([P, 96], F32, tag="blk")
                nc.vector.tensor_scalar(
                    out=blk[:, 0:T], in0=wb[:, 0:T],
                    scalar1=meta_f[:, i, 0:1], scalar2=None, op0=OP.mult)
                nc.vector.tensor_scalar(
                    out=blk[:, T:2 * T], in0=wb[:, T:2 * T],
                    scalar1=meta_f[:, i, 1:2], scalar2=None, op0=OP.mult)
                nc.vector.tensor_add(
                    out=blk[:, 0:2 * T], in0=blk[:, 0:2 * T], in1=phib[:])
                nc.scalar.activation(
                    out=blk[:, 0:2 * T], in_=blk[:, 0:2 * T], func=AF.Sin)
                nc.vector.tensor_copy(
                    out=blk[:, 2 * T:2 * T + NTS],
                    in_=meta_f[:, i, 2:3].to_broadcast([P, NTS]))
                ordrep = orp.tile([P, P], F32, tag="ordrep")
                nc.vector.tensor_copy(
                    out=ordrep[:],
                    in_=meta_f[:, i, 3:4].to_broadcast([P, P]))

                # ---- transposes (into the tail of the psa bank) ----
                psa = psap.tile([P, 1024], F32, tag="psa", space="PSUM")
                nc.tensor.transpose(
                    out=psa[0:2 * T + NTS, 768:896],
                    in_=blk[:, 0:2 * T + NTS], identity=id32[:])
                nc.tensor.transpose(
                    out=psa[:, 896:1024], in_=ordrep[:], identity=id32[:])
                sblk = sbp.tile([2 * T + NTS, P], F16, tag="sblk")
                nc.vector.tensor_copy(
                    out=sblk[:], in_=psa[0:2 * T + NTS, 768:896])
                ordt = otp.tile([P, P], F16, tag="ordt")
                nc.vector.tensor_copy(out=ordt[:], in_=psa[:, 896:1024])

                # ---- one-hots (fp16, exact for small ints) ----
                oh = ohp.tile([P, 4, P], F16, tag="oh")
                for c in range(4):
                    nc.vector.tensor_scalar(
                        out=oh[:, c, :], in0=ordt[:],
                        scalar1=iord[:, c:c + 1], scalar2=None,
                        op0=OP.is_equal)
                ohts = ohp.tile([NTS, P], F16, tag="ohts")
                nc.vector.tensor_scalar(
                    out=ohts[:], in0=sblk[2 * T:2 * T + NTS, :],
                    scalar1=its[:], scalar2=None, op0=OP.is_equal)

                # ---- pre-tanh: psa = W2[ids] + sinsT.T @ Ls ----
                nc.tensor.matmul(out=psa[:, 0:512], lhsT=id16[:],
                                 rhs=wg[:, 0:512], start=True, stop=False)
                nc.tensor.matmul(out=psa[:, 512:768], lhsT=id16[:],
                                 rhs=wg[:, 512:768], start=True, stop=False)
                nc.tensor.matmul(out=psa[:, 0:512], lhsT=sblk[0:2 * T, :],
                                 rhs=ls[:, 0:512], start=False, stop=True)
                nc.tensor.matmul(out=psa[:, 512:768], lhsT=sblk[0:2 * T, :],
                                 rhs=ls[:, 512:768], start=False, stop=True)

                # ---- tanh (+ running sum for LN mean) ----
                th = thp.tile([P, H + 1], F16, tag="th")
                sum_th = scl.tile([P, 1], F32, tag="sum_th")
                nc.vector.memset(th[:, H:H + 1], 0.0)
                nc.scalar.activation(
                    out=th[:, 0:H], in_=psa[:, 0:768], func=AF.Tanh,
                    accum_out=sum_th[:])

                # ---- post-tanh adds: order + (type,seg) tables + tanh ----
                psb = psbp.tile([P, 1024], F32, tag="psb", space="PSUM")
                for c in range(4):
                    nc.tensor.matmul(out=psb[:, 0:512], lhsT=oh[:, c, :],
                                     rhs=wcat[:, c, 0:512],
                                     start=(c == 0), stop=False)
                    nc.tensor.matmul(out=psb[:, 512:769], lhsT=oh[:, c, :],
                                     rhs=wcat[:, c, 512:769],
                                     start=(c == 0), stop=False)
                nc.tensor.matmul(out=psb[:, 0:512], lhsT=ohts[:],
                                 rhs=wts[:, 0:512], start=False, stop=False)
                nc.tensor.matmul(out=psb[:, 512:769], lhsT=ohts[:],
                                 rhs=wts[:, 512:769], start=False, stop=False)
                nc.tensor.matmul(out=psb[:, 0:512], lhsT=id16[:],
                                 rhs=th[:, 0:512], start=False, stop=True)
                nc.tensor.matmul(out=psb[:, 512:769], lhsT=id16[:],
                                 rhs=th[:, 512:769], start=False, stop=True)

                # ---- LN stats: E[x^2] via Square accum, mean via sums ----
                sq = sqp.tile([P, H], F16, tag="sq")
                sumsq = scl.tile([P, 1], F32, tag="sumsq")
                nc.scalar.activation(
                    out=sq[:], in_=psb[:, 0:768], func=AF.Square,
                    accum_out=sumsq[:])
                musum = scl.tile([P, 1], F32, tag="musum")
                nc.vector.tensor_add(
                    out=musum[:], in0=sum_th[:], in1=psb[:, 768:769])
                mu = scl.tile([P, 1], F32, tag="mu")
                nc.vector.tensor_scalar(
                    out=mu[:], in0=musum[:], scalar1=1.0 / H, scalar2=None,
                    op0=OP.mult)
                nmu2 = scl.tile([P, 1], F32, tag="nmu2")
                nc.vector.tensor_scalar(
                    out=nmu2[:], in0=mu[:], scalar1=mu[:], scalar2=-1.0,
                    op0=OP.mult, op1=OP.mult)
                sd = scl.tile([P, 1], F32, tag="sd")
                nc.scalar.activation(
                    out=sd[:], in_=sumsq[:], func=AF.Sqrt,
                    bias=nmu2[:], scale=1.0 / H)
                rstd = scl.tile([P, 1], F32, tag="rstd")
                nc.vector.reciprocal(out=rstd[:], in_=sd[:])

                # ---- normalize into the staged output ----
                if i % OB == 0:
                    ostage = ostp.tile([P, OB, H], F32, tag="ostage")
                nc.vector.tensor_scalar(
                    out=ostage[:, i % OB, :], in0=psb[:, 0:768],
                    scalar1=mu[:], scalar2=rstd[:],
                    op0=OP.subtract, op1=OP.mult)
                if apply_gb:
                    nc.vector.tensor_mul(out=ostage[:, i % OB, :],
                                         in0=ostage[:, i % OB, :], in1=g_sb[:])
                    nc.vector.tensor_add(out=ostage[:, i % OB, :],
                                         in0=ostage[:, i % OB, :], in1=b_sb[:])
                if i % OB == OB - 1:
                    nc.sync.dma_start(
                        out=out_d[:, (i - OB + 1) * H:(i + 1) * H],
                        in_=ostage[:, :, :])

    nc.finalize()
    return nc


def _prepare(inputs):
    f32c = lambda x: np.ascontiguousarray(np.asarray(x, dtype=np.float32))
    ids = np.asarray(inputs["input_ids"], dtype=np.int32)
    typ = np.asarray(inputs["type_ids"], dtype=np.int32)
    order = np.asarray(inputs["visit_orders"], dtype=np.int32)
    seg = np.asarray(inputs["visit_segments"], dtype=np.int32)
    ts = f32c(inputs["time_stamps"])
    ages = f32c(inputs["ages"])

    # dt[b, 0] = 0, dt[b, s] = ts[b, s] - ts[b, s-1] (matches reference)
    dt = np.concatenate([ts[:, :1] * 0.0, ts[:, 1:] - ts[:, :-1]], axis=1)
    tsidx = typ * SEGS + seg

    lin_W = f32c(inputs["lin_W"])
    lin_b = f32c(inputs["lin_b"])
    # cached projected word table (exact algebra; fp16 storage)
    W2 = (f32c(inputs["W_word"]) @ lin_W[:H] + lin_b).astype(np.float16)

    wo16 = f32c(inputs["W_order"]).astype(np.float16)
    wcat = np.concatenate(
        [wo16, wo16.astype(np.float32).sum(1, keepdims=True).astype(np.float16)],
        axis=1)
    t27 = (f32c(inputs["W_type"])[:, None, :]
           + f32c(inputs["W_seg"])[None, :, :]).reshape(NTS, H).astype(np.float16)
    wts = np.concatenate(
        [t27, t27.astype(np.float32).sum(1, keepdims=True).astype(np.float16)],
        axis=1)
    ls16 = lin_W[H:].astype(np.float16)

    wrow = np.concatenate([f32c(inputs["time_w"])[0], f32c(inputs["age_w"])[0]])
    prow = np.concatenate([f32c(inputs["time_phi"])[0], f32c(inputs["age_phi"])[0]])
    common = dict(
        W2=np.ascontiguousarray(W2).view(np.float32),
        Wcat=np.ascontiguousarray(wcat),
        Wts=np.ascontiguousarray(wts),
        Ls=np.ascontiguousarray(ls16),
        id32=np.eye(P, dtype=np.float32),
        id16=np.eye(P, dtype=np.float16),
        wb=np.ascontiguousarray(np.broadcast_to(wrow, (P, 2 * T)), dtype=np.float32),
        phib=np.ascontiguousarray(np.broadcast_to(prow, (P, 2 * T)), dtype=np.float32),
        iord=np.ascontiguousarray(
            (np.arange(P)[:, None] + P * np.arange(4)[None, :]).astype(np.float32)),
        its=np.arange(NTS, dtype=np.float32)[:, None].copy(),
    )

    ln_g = f32c(inputs["ln_g"])
    ln_beta = f32c(inputs["ln_beta"])
    apply_gb = not (np.all(ln_g == 1.0) and np.all(ln_beta == 0.0))
    if apply_gb:
        common["ln_g"] = np.ascontiguousarray(np.broadcast_to(ln_g, (P, H)))
        common["ln_beta"] = np.ascontiguousarray(np.broadcast_to(ln_beta, (P, H)))

    def tilemaj(a):
        # [TOK] -> [P, NTILES]: column t holds tokens t*128 .. t*128+127
        return np.ascontiguousarray(a.reshape(NTILES, P).T)

    in_maps = []
    for k in range(N_CORES):
        rows = slice(k * B_PER, (k + 1) * B_PER)
        m = dict(common)
        m["meta_i"] = tilemaj(ids[rows].reshape(TOK))
        mf = np.stack([dt[rows].reshape(TOK), ages[rows].reshape(TOK),
                       tsidx[rows].reshape(TOK).astype(np.float32),
                       order[rows].reshape(TOK).astype(np.float32)], axis=1)
        m["meta_f"] = np.ascontiguousarray(
            mf.reshape(NTILES, P, 4).transpose(1, 0, 2))
        in_maps.append(m)
    return in_maps, apply_gb


def run(inputs, trace=False):
    in_maps, apply_gb = _prepare(inputs)
    nc = build_nc(apply_gb)
    res = run_bass_kernel_spmd(nc, in_maps, list(range(N_CORES)), trace=trace)
    shards = [
        res.results[k]["out"].reshape(P, NTILES, H).transpose(1, 0, 2)
        .reshape(B_PER, S, H)
        for k in range(N_CORES)
    ]
    out = np.concatenate(shards, axis=0)
    return out, res


def kernel(**inputs) -> np.ndarray:
    out, _ = run(inputs, trace=False)
    return out
